# revision 60
# baseline (speedup 1.0000x reference)
"""Trainium2 Bass kernel for nn_BlastLinear (block low-rank linear layer).

Math (reference):
  y[q,n,r] = sum_c x[n, q*1024+c] * C[q,r,c]          (mm1, per input block q)
  z[p,n,r] = sum_q D[p,q,r] * y[q,n,r]                (tiny mix over q)
  o[p,n,j] = sum_r z[p,n,r] * B[p,j,r]                (mm2, per output block p)
  out[n, p*1024+j] = o[p,n,j] + bias[p*1024+j]

Sharding: pure data-parallel over the 8192 tokens -> 1024 tokens per core,
weights replicated, no collectives.

Precision: single-pass fp16 matmuls with fp32 PSUM accumulation. The
rel-err budget is 2e-2; fp16 operands give ~5e-4, so one pass suffices
(vs the old 3-pass f32r scheme) and cuts PE work 3x. All matmul operands
(x, C, B, z) ship/stay in fp16, halving HBM traffic too.

Per-core pipeline (chunk = 512 tokens, 2 chunks; order mm1(j0), mm1(j1),
mm2(j0), mm2(j1) so the DVE mix for chunk j overlaps mm1 of chunk j+1):
  warm: dummy matmuls bridge the PE p-state ramp while input DMAs fly
  mm1:  psum y^T[q,rt] [128r x 512n] += cth^T @ x    (PE, fp16, 1 pass;
        k-major first half + per-rt tails so banks stop staggered)
  copy: ACT copies each y psum bank to SBUF fp16     (frees banks fast;
        GPSIMD cannot read PSUM on real HW, so ACT does all of these)
  mix:  z[p,rt] = sum_q D[p,q,rt] * y_sb[q,rt]       (fp16 SBUF ops:
        q0 scale on GPSIMD; DVE tensor_scalar_mul at 4x rate +
        tensor_tensor adds at 2x rate for q1-3)
  mm2:  psum o^T[of, n] += sum_rt bth[p,rt,of]^T @ z[p,rt]  (PE, weights
        stationary; output is [out-feature partitions x tokens] so bias
        becomes a per-partition scalar)
  out:  ACT (DVE for odd tiles of the last chunk) drains o^T psum + bias
        -> fp16 SBUF, DMA to a transposed DRAM output outT[OUT_F,
        n_core]; host re-transposes. The very last tile is split into
        column halves with separate psum tiles/drains/DMAs so the final
        drain->DMA->sem tail starts before PE finishes.

Engine budget per core (TimelineSim cost model): PE 512 matmuls x 213ns
= 109us (bottleneck, zero stalls after warmup), ACT ~60us
(copies+drains), DVE ~56us (mix), Pool ~14us (q0 scale), DMA ~70us
(24 MiB at 360 GB/s) -- all overlapped under PE. Modeled exec ~117.9us
vs the 365.8us 3-pass-f32r baseline.
"""

import numpy as np

import concourse.mybir as mybir
import concourse.tile as tile
from concourse import bacc
from concourse.bass_utils import run_bass_kernel_spmd

N_CORES = 8
IN_F = 4096
OUT_F = 4096
P = 4
Q = 4
R = 512
CB = IN_F // Q        # 1024 input features per q block
OB = OUT_F // P       # 1024 output features per p block
N_TOK = 4 * 2048      # 8192 total tokens
N_CORE = N_TOK // N_CORES   # 1024 tokens per core

CHUNK = 512           # tokens per pipeline chunk
KT1 = CB // 128       # 8 contraction tiles per q in mm1
RT = R // 128         # 4 rank partition tiles
OF = OB // 128        # 8 out-feature tiles per p in mm2

F32 = mybir.dt.float32
F16 = mybir.dt.float16
ADD = mybir.AluOpType.add

_cached_nc = None


def _build(n_core=N_CORE, chunk=CHUNK):
    nc = bacc.Bacc("TRN2", target_bir_lowering=False, debug=False,
                   enable_asserts=False)

    def din(name, shape, dtype):
        return nc.dram_tensor(name, shape, dtype, kind="ExternalInput").ap()

    xt = din("xt", [IN_F, n_core], F16)        # x^T, feature-major
    ct = din("ct", [IN_F, R], F16)             # C^T rows c = q*1024+c_local
    bt = din("bt", [P * R, OB], F16)           # B^T rows r = p*512+r_local
    dr = din("dr", [R, P * Q], F32)            # D[r, p*4+q]
    biasT = din("biasT", [128, OUT_F // 128], F32)  # bias[(p*8+of)*128+i] at [i, p*8+of]
    boot = din("boot", [128, 2, chunk], F16)   # [cth k-tile 0 | x k0 chunk 0]
    outT = nc.dram_tensor("outT", [OUT_F, n_core], F16,
                          kind="ExternalOutput").ap()

    n_chunks = n_core // chunk

    with tile.TileContext(nc) as tc:
        with (
            tc.tile_pool(name="wc", bufs=1) as wc,
            tc.tile_pool(name="ysb", bufs=16) as ysbpool,
            tc.tile_pool(name="tp", bufs=6) as tpool,
            tc.tile_pool(name="zp", bufs=16 * n_chunks) as zpool,
            tc.tile_pool(name="otp", bufs=4) as otpool,
            tc.tile_pool(name="yps", bufs=5, space="PSUM") as ypool,
            tc.tile_pool(name="ops", bufs=3, space="PSUM") as opool,
        ):
            cth_sb = wc.tile([128, IN_F // 128, R], F16)
            x_sb = wc.tile([128, IN_F // 128, n_core], F16)
            bth_sb = wc.tile([128, P * RT, OB], F16)
            d_sb = wc.tile([128, RT, P * Q], F32)
            bias_sb = wc.tile([128, OUT_F // 128], F32)
            boot_sb = wc.tile([128, 2, chunk], F16)

            cth3 = ct.rearrange("(t p) r -> p t r", p=128)
            xr = xt.rearrange("(t p) n -> p t n", p=128)
            btr = bt.rearrange("(t p) o -> p t o", p=128)

            z = {}

            def emit_warmup():
                # Tiny dummy matmuls on a zeroed tile keep PE continuously
                # busy from ~1us while the first input DMAs are in flight,
                # so the p-state ramp (cold 1.2GHz -> warm 2.4GHz after
                # 3us of sustained activity) completes before real work
                # and the early matmuls aren't charged at cold rates.
                warm = wc.tile([128, 128], F16)
                nc.vector.memset(warm[:], 0.0)
                wps = ypool.tile([128, chunk], F32, tag="y", name="warmps")
                for _ in range(62):
                    nc.tensor.matmul(wps[:, 0:64], lhsT=warm[:],
                                     rhs=warm[:, 0:64],
                                     start=True, stop=True)

            def emit_input_dmas():
                # chunk-0 essentials stream first, one (cth, x) pair per q
                # phase (5.8us DMA per 6.8us of PE) so mm1(j0) never
                # starves; chunk-1 x and the mm2 weights follow during
                # mm1(j1). The first k-tile of q0 rides ahead so matmuls
                # start ASAP.
                c0 = slice(0, chunk)
                # the host packs (cth k0, x k0) into one boot tensor; it
                # rides Pool's SWDGE path, skipping the serialized HWDGE
                # queue, so the first real matmul's data lands ~0.9us
                # earlier than via the SP stream
                nc.gpsimd.dma_start(boot_sb[:], boot[:])
                for q in range(Q):
                    # 2-ktile (cth, x) pairs so every k-tile has both its
                    # weights and activations shortly before PE reaches it
                    for h in range(0, KT1, 2):
                        s = slice(q * KT1 + h, q * KT1 + h + 2)
                        if q == 0 and h == 0:
                            s = slice(1, 2)
                        nc.sync.dma_start(cth_sb[:, s, :], cth3[:, s, :])
                        nc.sync.dma_start(x_sb[:, s, c0], xr[:, s, c0])
                    if q == 0:
                        # needed by the first mix, ~11us in
                        nc.sync.dma_start(
                            d_sb[:], dr.rearrange("(t p) s -> p t s", p=128))
                    elif q == 1:
                        # needed by the first mm2 drain, ~60us in
                        nc.sync.dma_start(bias_sb[:], biasT)
                for j in range(1, n_chunks):
                    cj = slice(j * chunk, (j + 1) * chunk)
                    for q in range(Q):
                        s = slice(q * KT1, (q + 1) * KT1)
                        nc.sync.dma_start(x_sb[:, s, cj], xr[:, s, cj])
                nc.sync.dma_start(bth_sb[:], btr)

            def emit_mm1(j):
                for q in range(Q):
                    # First half k-major (input consumption tracks the DMA
                    # stream), second half per-rt tails so each y bank's
                    # accumulation STOPS early and staggered -- its freeing
                    # copy (split ACT/Pool) then completes well before the
                    # next q phase reuses the bank, with zero PE stalls
                    # even at 5 y banks.
                    ys = [
                        ypool.tile([128, chunk], F32, tag="y",
                                   name=f"y_{j}_{q}_{rt}")
                        for rt in range(RT)
                    ]
                    half = KT1 // 2
                    cslice = slice(j * chunk, (j + 1) * chunk)

                    def mm(rt, k):
                        kk = q * KT1 + k
                        if kk == 0:
                            # k-tile 0 lives in the boot tensor (cth row
                            # block + chunk-0 x block packed by the host)
                            lhsT = boot_sb[:, 0, rt * 128:(rt + 1) * 128]
                            rhs = (boot_sb[:, 1, :] if j == 0
                                   else x_sb[:, 0, cslice])
                        else:
                            lhsT = cth_sb[:, kk, rt * 128:(rt + 1) * 128]
                            rhs = x_sb[:, kk, cslice]
                        nc.tensor.matmul(
                            ys[rt][:], lhsT=lhsT, rhs=rhs,
                            start=(k == 0), stop=(k == KT1 - 1))

                    for k in range(half):
                        for rt in range(RT):
                            mm(rt, k)
                    for rt in range(RT):
                        for k in range(half, KT1):
                            mm(rt, k)
                    for rt in range(RT):
                        ysb = ysbpool.tile([128, chunk], F16, tag="ysb",
                                           name=f"ysb_{j}_{q}_{rt}")
                        # PSUM is only readable by ACT/DVE on real HW
                        # (GPSIMD cannot access PSUM); the staggered stops
                        # from the hybrid mm1 order give the serial ACT
                        # copies plenty of slack before banks are reused.
                        nc.scalar.copy(ysb[:], ys[rt][:])
                        for p in range(P):
                            col = p * Q + q
                            dcol = d_sb[:, rt, col:col + 1]
                            if q == 0:
                                # first mix layer runs off-DVE (ACT/Pool)
                                # to keep the saturated DVE under its
                                # per-phase budget
                                zt = zpool.tile([128, chunk], F16, tag="z",
                                                name=f"z_{j}_{p}_{rt}")
                                z[(j, p, rt)] = zt
                                # SBUF-only op: offload to the idle GPSIMD
                                nc.gpsimd.tensor_scalar_mul(
                                    zt[:], ysb[:], dcol)
                            else:
                                tmp = tpool.tile([128, chunk], F16, tag="t",
                                                 name=f"t_{j}_{q}_{p}_{rt}")
                                nc.vector.tensor_scalar_mul(
                                    tmp[:], ysb[:], dcol)
                                zt = z[(j, p, rt)]
                                nc.vector.tensor_tensor(
                                    zt[:], zt[:], tmp[:], op=ADD)

            def emit_mm2(j, last=False):
                for p in range(P):
                    for of in range(OF):
                        rows = slice(p * OB + of * 128,
                                     p * OB + (of + 1) * 128)
                        idx = p * OF + of
                        bcol = bias_sb[:, idx:idx + 1]
                        if last and p == P - 1 and of == OF - 1:
                            # split the final tile into column halves with
                            # separate psum tiles, drain targets and DMAs,
                            # so the kernel's tail (drain -> DMA chain of
                            # the very last data) starts ~850ns before PE
                            # finishes
                            h = chunk // 2
                            for hi, (eng, tg) in enumerate((
                                    (nc.vector.tensor_scalar_add, "ota"),
                                    (nc.vector.tensor_scalar_add, "otb"))):
                                cs = slice(hi * h, (hi + 1) * h)
                                # y banks are idle during mm2 -- borrow
                                # their pool so the final halves don't
                                # wait on the o-ring recycling
                                o_h = ypool.tile([128, h], F32, tag="y",
                                                 name=f"oh_{hi}")
                                for rt in range(RT):
                                    nc.tensor.matmul(
                                        o_h[:],
                                        lhsT=bth_sb[:, p * RT + rt,
                                                    of * 128:(of + 1) * 128],
                                        rhs=z[(j, p, rt)][:, cs],
                                        start=(rt == 0), stop=(rt == RT - 1))
                                oth = otpool.tile([128, h], F16, tag=tg,
                                                  name=f"{tg}_{j}")
                                eng(oth[:], o_h[:], bcol)
                                # first half's DMA bypasses the HWDGE queue
                                # via Pool's SWDGE path; the second rides SP
                                dma = (nc.gpsimd.dma_start if hi == 0
                                       else nc.sync.dma_start)
                                dma(outT[rows, j * chunk + hi * h:
                                         j * chunk + (hi + 1) * h],
                                    oth[:])
                            continue
                        o_ps = opool.tile([128, chunk], F32, tag="o",
                                          name=f"o_{j}_{p}_{of}")
                        for rt in range(RT):
                            nc.tensor.matmul(
                                o_ps[:],
                                lhsT=bth_sb[:, p * RT + rt,
                                            of * 128:(of + 1) * 128],
                                rhs=z[(j, p, rt)][:],
                                start=(rt == 0), stop=(rt == RT - 1))
                        ot = otpool.tile([128, chunk], F16, tag="ot",
                                         name=f"ot_{j}_{p}_{of}")
                        if last and of % 2 == 1:
                            # in the last chunk the DVE mix is long done,
                            # so alternating drains ACT/DVE is safe and
                            # halves the ACT backlog at the kernel tail
                            nc.vector.tensor_scalar_add(ot[:], o_ps[:], bcol)
                        else:
                            nc.scalar.add(ot[:], o_ps[:], bcol)
                        nc.sync.dma_start(
                            outT[rows, j * chunk:(j + 1) * chunk],
                            ot[:])

            emit_warmup()
            emit_input_dmas()
            for j in range(n_chunks):
                emit_mm1(j)
            for j in range(n_chunks):
                emit_mm2(j, last=(j == n_chunks - 1))

    nc.compile()
    return nc


def _prep_in_maps(x, B, C, D, bias):
    x2 = np.asarray(x, dtype=np.float32).reshape(N_TOK, IN_F)
    CT = np.asarray(C, dtype=np.float32).transpose(0, 2, 1).reshape(
        IN_F, R).astype(np.float16)
    BT = np.asarray(B, dtype=np.float32).transpose(0, 2, 1).reshape(
        P * R, OB).astype(np.float16)
    DR = np.ascontiguousarray(
        np.asarray(D, dtype=np.float32).transpose(2, 0, 1).reshape(R, P * Q))
    BIAS_T = np.ascontiguousarray(
        np.asarray(bias, dtype=np.float32).reshape(OUT_F // 128, 128).T)

    in_maps = []
    for c in range(N_CORES):
        xtc = np.ascontiguousarray(
            x2[c * N_CORE:(c + 1) * N_CORE].T.astype(np.float16))
        boot = np.ascontiguousarray(
            np.stack([CT[0:128, :], xtc[0:128, 0:CHUNK]], axis=1))
        in_maps.append({
            "xt": xtc, "ct": CT, "bt": BT,
            "dr": DR, "biasT": BIAS_T, "boot": boot,
        })
    return in_maps


def _run(in_maps, trace=False):
    global _cached_nc
    if _cached_nc is None:
        _cached_nc = _build()
    import time
    for attempt in range(3):
        try:
            return run_bass_kernel_spmd(
                _cached_nc, in_maps, list(range(N_CORES)), trace=trace)
        except Exception:
            # transient device errors (e.g. NRT_EXEC_UNIT_UNRECOVERABLE
            # from a previously wedged core) usually clear on retry
            if attempt == 2:
                raise
            time.sleep(5.0 * (attempt + 1))


def kernel(x, B, C, D, bias):
    lead = np.asarray(x).shape[:-1]
    res = _run(_prep_in_maps(x, B, C, D, bias))
    out = np.empty((N_TOK, OUT_F), dtype=np.float32)
    for c in range(N_CORES):
        out[c * N_CORE:(c + 1) * N_CORE] = np.asarray(
            res.results[c]["outT"]).T.astype(np.float32)
    return out.reshape(*lead, OUT_F)
